# revision 11
# baseline (speedup 1.0000x reference)
"""Llama4TextMoe (T=4096, H=2048, I=1024, E=16, top-1) on 8 trn2 NeuronCores.

Strategy (expert-parallel, sparse dispatch):
  - Host: router argmax (integer routing decision only), balanced expert->core
    pairing (2 experts/core), gather of each expert's tokens, transposition to
    feature-major layout, bf16 casts, final scatter-add combine.
  - Device (SPMD, same program, per-core data): router scores
    (matvec + sigmoid), score scaling, SwiGLU expert FFN for the core's two
    experts over only their routed tokens, plus the shared-expert SwiGLU for
    this core's 512-token block. All matmuls bf16 with fp32 PSUM accumulation,
    feature-major activations so weights stream as natural-layout lhsT tiles.
  - Combine: out[block] = shared_block.T ; out[idx_e] += routed_e.T (host).
"""

import os
import sys

import numpy as np
import ml_dtypes

for _p in ("/opt/trn_rl_repo", "/root/.axon_site/_ro/trn_rl_repo"):
    if _p not in sys.path and os.path.isdir(_p):
        sys.path.append(_p)

T, H, I, E = 4096, 2048, 1024, 16
NCORES = 8
TOK_BLK = T // NCORES  # 512
KH = H // 128   # 16 contraction tiles over H
KI = I // 128   # 8  contraction tiles over I
MF = (2 * I) // 128  # 16 output tiles over gate_up ff dim
MH = H // 128   # 16 output tiles over H
BF16 = ml_dtypes.bfloat16

_PROG_CACHE: dict = {}


def _chunks(total, step):
    out = []
    st = 0
    while st < total:
        w = min(step, total - st)
        out.append((st, w))
        st += w
    return out


def _build_program(cap):
    import concourse.mybir as mybir
    import concourse.tile as tile
    from concourse import bacc

    f32 = mybir.dt.float32
    bf16 = mybir.dt.bfloat16
    AF = mybir.ActivationFunctionType
    MULT = mybir.AluOpType.mult
    NR = 2 * cap
    assert cap <= 512, f"per-expert capacity {cap} exceeds one PSUM bank"

    nc = bacc.Bacc("TRN2", target_bir_lowering=False, debug=False,
                   num_devices=NCORES)
    xg_d = nc.dram_tensor("xg", [H, NR], bf16, kind="ExternalInput").ap()
    rw_d = nc.dram_tensor("rw", [128, 2 * KH], bf16, kind="ExternalInput").ap()
    wgu_d = nc.dram_tensor("wgu", [2, H, 2 * I], bf16, kind="ExternalInput").ap()
    wd_d = nc.dram_tensor("wd", [2, I, H], bf16, kind="ExternalInput").ap()
    xs_d = nc.dram_tensor("xs", [H, TOK_BLK], bf16, kind="ExternalInput").ap()
    wsgu_d = nc.dram_tensor("wsgu", [H, 2 * I], bf16, kind="ExternalInput").ap()
    wsd_d = nc.dram_tensor("wsd", [I, H], bf16, kind="ExternalInput").ap()
    yr_d = nc.dram_tensor("yr", [H, NR], f32, kind="ExternalOutput").ap()
    ys_d = nc.dram_tensor("ys", [H, TOK_BLK], f32, kind="ExternalOutput").ap()

    with tile.TileContext(nc) as tc:
        with (
            tc.tile_pool(name="xgb", bufs=KH) as xgb_pool,
            tc.tile_pool(name="xgs", bufs=KH) as xgs_pool,
            tc.tile_pool(name="xsb", bufs=KH) as xsb_pool,
            tc.tile_pool(name="wslab", bufs=50) as w_pool,
            tc.tile_pool(name="act", bufs=10) as act_pool,
            tc.tile_pool(name="ga", bufs=8) as ga_pool,
            tc.tile_pool(name="up", bufs=2) as up_pool,
            tc.tile_pool(name="ev", bufs=5) as ev_pool,
            tc.tile_pool(name="small", bufs=2) as small_pool,
            tc.tile_pool(name="pmm", bufs=4, space="PSUM") as pmm_pool,
            tc.tile_pool(name="psml", bufs=2, space="PSUM") as psml_pool,
        ):
            # --- router weights + ones (for partition-broadcast matmul) ---
            rwt = small_pool.tile([128, 2 * KH], bf16, tag="rwt")
            nc.sync.dma_start(rwt[:], rw_d[:])
            ones = small_pool.tile([1, 128], bf16, tag="ones")
            nc.gpsimd.memset(ones[:], 1.0)

            # --- shared-expert token block, bf16 feature-major (first:
            #     the shared FFN has no score dependency; it is the first
            #     phase and keeps PE busy while the score chain resolves) ---
            xsb = []
            for k in range(KH):
                t = xsb_pool.tile([128, TOK_BLK], bf16, tag="xsb")
                eng = nc.sync if k % 2 == 0 else nc.gpsimd
                eng.dma_start(t[:], xs_d[k * 128:(k + 1) * 128, :])
                xsb.append(t)

            def emit_scores():
                # --- gathered routed tokens, feature-major [H, NR] bf16 ---
                xgb = []
                for k in range(KH):
                    t = xgb_pool.tile([128, NR], bf16, tag="xgb")
                    nc.sync.dma_start(t[:], xg_d[k * 128:(k + 1) * 128, :])
                    xgb.append(t)

                # --- scores: s[j] = sigmoid(router_w[e_j] . x_t) ---
                s_sb = small_pool.tile([1, NR], bf16, tag="s_sb")
                for j in range(2):
                    ps = psml_pool.tile([1, cap], f32, tag="ps_score")
                    for k in range(KH):
                        nc.tensor.matmul(
                            ps[:], rwt[:, 2 * k + j:2 * k + j + 1],
                            xgb[k][:, j * cap:(j + 1) * cap],
                            start=(k == 0), stop=(k == KH - 1))
                    nc.scalar.activation(s_sb[:, j * cap:(j + 1) * cap],
                                         ps[:], AF.Sigmoid)

                # --- broadcast scores over 128 partitions via ones-matmul ---
                sbc = small_pool.tile([128, NR], bf16, tag="sbc")
                for (st, w) in _chunks(NR, 512):
                    pb = psml_pool.tile([128, w], f32, tag="pb")
                    nc.tensor.matmul(pb[:], ones[:], s_sb[:, st:st + w],
                                     start=True, stop=True)
                    nc.vector.tensor_copy(sbc[:, st:st + w], pb[:])

                # --- routed_in = x * score (both segments at once) ---
                xgs = []
                for k in range(KH):
                    t = xgs_pool.tile([128, NR], bf16, tag="xgs")
                    nc.vector.tensor_tensor(t[:], xgb[k][:], sbc[:], MULT)
                    xgs.append(t)
                return xgs

            # Weight slabs are loaded as column-halves ([128, 1024]): the
            # first half's last use is m-tile 7, so it frees mid-phase and
            # the NEXT phase's weight DMAs start that much earlier.
            def swiglu_ffn(x_tiles, n_tok, seg_off, gu_src, d_src, y_ap, y_off):
                """gate_up -> silu*up -> down, feature-major, N=n_tok tokens."""
                guslab = [[None] * KH for _ in range(2)]
                for h in range(2):
                    for k in range(KH):
                        t = w_pool.tile([128, I], bf16, tag="wslab")
                        eng = nc.sync if k % 2 == 0 else nc.gpsimd
                        eng.dma_start(t[:], gu_src(k, h))
                        guslab[h][k] = t
                ga = [None] * KI
                at = [None] * KI
                for m in range(MF):
                    pg = pmm_pool.tile([128, n_tok], f32, tag="pmm")
                    mh, mo = divmod(m, KI)
                    for k in range(KH):
                        nc.tensor.matmul(
                            pg[:], guslab[mh][k][:, mo * 128:(mo + 1) * 128],
                            x_tiles[k][:, seg_off:seg_off + n_tok],
                            start=(k == 0), stop=(k == KH - 1))
                    if m < KI:  # gate half
                        g = ga_pool.tile([128, n_tok], bf16, tag="ga")
                        nc.scalar.activation(g[:], pg[:], AF.Silu)
                        ga[m] = g
                    else:       # up half -> act = silu(gate) * up
                        u = up_pool.tile([128, n_tok], bf16, tag="up")
                        nc.vector.tensor_copy(u[:], pg[:])
                        a = act_pool.tile([128, n_tok], bf16, tag="at")
                        nc.vector.tensor_tensor(a[:], ga[m - KI][:], u[:], MULT)
                        at[m - KI] = a
                dslab = [[None] * KI for _ in range(2)]
                for h in range(2):
                    for ki in range(KI):
                        t = w_pool.tile([128, I], bf16, tag="wslab")
                        eng = nc.sync if ki % 2 == 0 else nc.gpsimd
                        eng.dma_start(t[:], d_src(ki, h))
                        dslab[h][ki] = t
                for m2 in range(MH):
                    pd = pmm_pool.tile([128, n_tok], f32, tag="pmm")
                    m2h, m2o = divmod(m2, KI)
                    for ki in range(KI):
                        nc.tensor.matmul(
                            pd[:], dslab[m2h][ki][:, m2o * 128:(m2o + 1) * 128],
                            at[ki][:],
                            start=(ki == 0), stop=(ki == KI - 1))
                    ev = ev_pool.tile([128, n_tok], f32, tag="ev")
                    nc.vector.tensor_copy(ev[:], pd[:])
                    nc.sync.dma_start(
                        y_ap[m2 * 128:(m2 + 1) * 128, y_off:y_off + n_tok],
                        ev[:])

            swiglu_ffn(
                xsb, TOK_BLK, 0,
                lambda k, h: wsgu_d[k * 128:(k + 1) * 128,
                                    h * I:(h + 1) * I],
                lambda ki, h: wsd_d[ki * 128:(ki + 1) * 128,
                                    h * I:(h + 1) * I],
                ys_d, 0)
            xgs = emit_scores()
            for j in range(2):
                swiglu_ffn(
                    xgs, cap, j * cap,
                    lambda k, h, j=j: wgu_d[j, k * 128:(k + 1) * 128,
                                            h * I:(h + 1) * I],
                    lambda ki, h, j=j: wd_d[j, ki * 128:(ki + 1) * 128,
                                            h * I:(h + 1) * I],
                    yr_d, j * cap)

    nc.compile()
    return nc


def _plan(hidden_states, router_w):
    logits = hidden_states.astype(np.float64) @ router_w.astype(np.float64).T
    top = np.argmax(logits, axis=1)
    counts = np.bincount(top, minlength=E)
    order = np.argsort(-counts, kind="stable")
    pairs = [(int(order[i]), int(order[E - 1 - i])) for i in range(NCORES)]
    cap = int(np.ceil(max(1, counts.max()) / 8) * 8)
    idx = [np.nonzero(top == e)[0] for e in range(E)]
    return pairs, cap, idx


def _run(inputs, trace=False):
    from concourse.bass_utils import run_bass_kernel_spmd

    x = np.asarray(inputs["hidden_states"], dtype=np.float32)
    router_w = np.asarray(inputs["router_w"], dtype=np.float32)
    gup = np.asarray(inputs["gate_up_proj"], dtype=np.float32)
    dp = np.asarray(inputs["down_proj"], dtype=np.float32)
    sgu = np.asarray(inputs["shared_gate_up_w"], dtype=np.float32)
    sd = np.asarray(inputs["shared_down_w"], dtype=np.float32)

    pairs, cap, idx = _plan(x, router_w)
    NR = 2 * cap

    key = cap
    if key not in _PROG_CACHE:
        _PROG_CACHE[key] = _build_program(cap)
    nc = _PROG_CACHE[key]

    xT = np.ascontiguousarray(x.T)                     # [H, T] f32
    xT_bf = xT.astype(BF16)
    wsgu_t = np.ascontiguousarray(sgu.T).astype(BF16)  # [H, 2I]
    wsd_t = np.ascontiguousarray(sd.T).astype(BF16)    # [I, H]

    in_maps = []
    for c in range(NCORES):
        eA, eB = pairs[c]
        xg = np.zeros((H, NR), dtype=BF16)
        for j, e in enumerate((eA, eB)):
            ids = idx[e]
            xg[:, j * cap: j * cap + len(ids)] = xT_bf[:, ids]
        rw = np.stack([router_w[eA], router_w[eB]], axis=1)  # [H, 2]
        rw = rw.reshape(KH, 128, 2).transpose(1, 0, 2).reshape(128, 2 * KH)
        in_maps.append({
            "xg": xg,
            "rw": np.ascontiguousarray(rw).astype(BF16),
            "wgu": gup[[eA, eB]].astype(BF16),
            "wd": dp[[eA, eB]].astype(BF16),
            "xs": np.ascontiguousarray(
                xT_bf[:, c * TOK_BLK:(c + 1) * TOK_BLK]),
            "wsgu": wsgu_t,
            "wsd": wsd_t,
        })

    res = run_bass_kernel_spmd(nc, in_maps, core_ids=list(range(NCORES)),
                               trace=trace)

    out = np.empty((T, H), dtype=np.float32)
    for c in range(NCORES):
        out[c * TOK_BLK:(c + 1) * TOK_BLK] = res.results[c]["ys"].T
    for c in range(NCORES):
        yr = res.results[c]["yr"]
        for j, e in enumerate(pairs[c]):
            ids = idx[e]
            out[ids] += yr[:, j * cap: j * cap + len(ids)].T
    return out, res.exec_time_ns


def kernel(**inputs) -> np.ndarray:
    out, _ = _run(inputs, trace=False)
    return out


# revision 12
# speedup vs baseline: 1.0637x; 1.0637x over previous
"""Llama4TextMoe (T=4096, H=2048, I=1024, E=16, top-1) on 8 trn2 NeuronCores.

Strategy (expert-parallel, sparse dispatch):
  - Host: router argmax (integer routing decision only), balanced expert->core
    pairing (2 experts/core), gather of each expert's tokens, transposition to
    feature-major layout, bf16 casts, final scatter-add combine.
  - Device (SPMD, same program, per-core data): router scores
    (matvec + sigmoid), score scaling, SwiGLU expert FFN for the core's two
    experts over only their routed tokens, plus the shared-expert SwiGLU for
    this core's 512-token block. All matmuls bf16 with fp32 PSUM accumulation,
    feature-major activations so weights stream as natural-layout lhsT tiles.
  - Combine: out[block] = shared_block.T ; out[idx_e] += routed_e.T (host).
"""

import os
import sys

import numpy as np
import ml_dtypes

for _p in ("/opt/trn_rl_repo", "/root/.axon_site/_ro/trn_rl_repo"):
    if _p not in sys.path and os.path.isdir(_p):
        sys.path.append(_p)

T, H, I, E = 4096, 2048, 1024, 16
NCORES = 8
TOK_BLK = T // NCORES  # 512
KH = H // 128   # 16 contraction tiles over H
KI = I // 128   # 8  contraction tiles over I
MF = (2 * I) // 128  # 16 output tiles over gate_up ff dim
MH = H // 128   # 16 output tiles over H
BF16 = ml_dtypes.bfloat16

_PROG_CACHE: dict = {}


def _chunks(total, step):
    out = []
    st = 0
    while st < total:
        w = min(step, total - st)
        out.append((st, w))
        st += w
    return out


def _build_program(cap):
    import concourse.mybir as mybir
    import concourse.tile as tile
    from concourse import bacc

    f32 = mybir.dt.float32
    bf16 = mybir.dt.bfloat16
    AF = mybir.ActivationFunctionType
    MULT = mybir.AluOpType.mult
    NR = 2 * cap
    assert cap <= 512, f"per-expert capacity {cap} exceeds one PSUM bank"

    nc = bacc.Bacc("TRN2", target_bir_lowering=False, debug=False,
                   num_devices=NCORES)
    xg_d = nc.dram_tensor("xg", [H, NR], bf16, kind="ExternalInput").ap()
    rw_d = nc.dram_tensor("rw", [128, 2 * KH], bf16, kind="ExternalInput").ap()
    wgu_d = nc.dram_tensor("wgu", [2, H, 2 * I], bf16, kind="ExternalInput").ap()
    wd_d = nc.dram_tensor("wd", [2, I, H], bf16, kind="ExternalInput").ap()
    xs_d = nc.dram_tensor("xs", [H, TOK_BLK], bf16, kind="ExternalInput").ap()
    wsgu_d = nc.dram_tensor("wsgu", [H, 2 * I], bf16, kind="ExternalInput").ap()
    wsd_d = nc.dram_tensor("wsd", [I, H], bf16, kind="ExternalInput").ap()
    yr_d = nc.dram_tensor("yr", [H, NR], f32, kind="ExternalOutput").ap()
    ys_d = nc.dram_tensor("ys", [H, TOK_BLK], f32, kind="ExternalOutput").ap()

    with tile.TileContext(nc) as tc:
        with (
            tc.tile_pool(name="xgb", bufs=KH) as xgb_pool,
            tc.tile_pool(name="xgs", bufs=KH) as xgs_pool,
            tc.tile_pool(name="xsb", bufs=KH) as xsb_pool,
            tc.tile_pool(name="wslab", bufs=50) as w_pool,
            tc.tile_pool(name="act", bufs=10) as act_pool,
            tc.tile_pool(name="ga", bufs=8) as ga_pool,
            tc.tile_pool(name="up", bufs=2) as up_pool,
            tc.tile_pool(name="ev", bufs=5) as ev_pool,
            tc.tile_pool(name="small", bufs=2) as small_pool,
            tc.tile_pool(name="pmm", bufs=4, space="PSUM") as pmm_pool,
            tc.tile_pool(name="psml", bufs=2, space="PSUM") as psml_pool,
        ):
            # --- router weights + ones (for partition-broadcast matmul) ---
            rwt = small_pool.tile([128, 2 * KH], bf16, tag="rwt")
            nc.sync.dma_start(rwt[:], rw_d[:])
            ones = small_pool.tile([1, 128], bf16, tag="ones")
            nc.gpsimd.memset(ones[:], 1.0)

            # --- shared-expert token block, bf16 feature-major (first:
            #     the shared FFN has no score dependency; it is the first
            #     phase and keeps PE busy while the score chain resolves) ---
            xsb = []
            for k in range(KH):
                t = xsb_pool.tile([128, TOK_BLK], bf16, tag="xsb")
                nc.sync.dma_start(t[:], xs_d[k * 128:(k + 1) * 128, :])
                xsb.append(t)

            def emit_scores():
                # --- gathered routed tokens, feature-major [H, NR] bf16 ---
                xgb = []
                for k in range(KH):
                    t = xgb_pool.tile([128, NR], bf16, tag="xgb")
                    nc.sync.dma_start(t[:], xg_d[k * 128:(k + 1) * 128, :])
                    xgb.append(t)

                # --- scores: s[j] = sigmoid(router_w[e_j] . x_t) ---
                s_sb = small_pool.tile([1, NR], bf16, tag="s_sb")
                for j in range(2):
                    ps = psml_pool.tile([1, cap], f32, tag="ps_score")
                    for k in range(KH):
                        nc.tensor.matmul(
                            ps[:], rwt[:, 2 * k + j:2 * k + j + 1],
                            xgb[k][:, j * cap:(j + 1) * cap],
                            start=(k == 0), stop=(k == KH - 1))
                    nc.scalar.activation(s_sb[:, j * cap:(j + 1) * cap],
                                         ps[:], AF.Sigmoid)

                # --- broadcast scores over 128 partitions via ones-matmul ---
                sbc = small_pool.tile([128, NR], bf16, tag="sbc")
                for (st, w) in _chunks(NR, 512):
                    pb = psml_pool.tile([128, w], f32, tag="pb")
                    nc.tensor.matmul(pb[:], ones[:], s_sb[:, st:st + w],
                                     start=True, stop=True)
                    nc.vector.tensor_copy(sbc[:, st:st + w], pb[:])

                # --- routed_in = x * score (both segments at once) ---
                xgs = []
                for k in range(KH):
                    t = xgs_pool.tile([128, NR], bf16, tag="xgs")
                    nc.vector.tensor_tensor(t[:], xgb[k][:], sbc[:], MULT)
                    xgs.append(t)
                return xgs

            # Weight slabs are loaded as column-halves ([128, 1024]): the
            # first half's last use is m-tile 7, so it frees mid-phase and
            # the NEXT phase's weight DMAs start that much earlier.
            def swiglu_ffn(x_tiles, n_tok, seg_off, gu_src, d_src, y_ap, y_off):
                """gate_up -> silu*up -> down, feature-major, N=n_tok tokens."""
                guslab = [[None] * KH for _ in range(2)]
                for h in range(2):
                    for k in range(KH):
                        t = w_pool.tile([128, I], bf16, tag="wslab")
                        nc.sync.dma_start(t[:], gu_src(k, h))
                        guslab[h][k] = t
                ga = [None] * KI
                at = [None] * KI
                for m in range(MF):
                    pg = pmm_pool.tile([128, n_tok], f32, tag="pmm")
                    mh, mo = divmod(m, KI)
                    for k in range(KH):
                        nc.tensor.matmul(
                            pg[:], guslab[mh][k][:, mo * 128:(mo + 1) * 128],
                            x_tiles[k][:, seg_off:seg_off + n_tok],
                            start=(k == 0), stop=(k == KH - 1))
                    if m < KI:  # gate half
                        g = ga_pool.tile([128, n_tok], bf16, tag="ga")
                        nc.scalar.activation(g[:], pg[:], AF.Silu)
                        ga[m] = g
                    else:       # up half -> act = silu(gate) * up
                        u = up_pool.tile([128, n_tok], bf16, tag="up")
                        nc.vector.tensor_copy(u[:], pg[:])
                        a = act_pool.tile([128, n_tok], bf16, tag="at")
                        nc.vector.tensor_tensor(a[:], ga[m - KI][:], u[:], MULT)
                        at[m - KI] = a
                dslab = [[None] * KI for _ in range(2)]
                for h in range(2):
                    for ki in range(KI):
                        t = w_pool.tile([128, I], bf16, tag="wslab")
                        nc.sync.dma_start(t[:], d_src(ki, h))
                        dslab[h][ki] = t
                for m2 in range(MH):
                    pd = pmm_pool.tile([128, n_tok], f32, tag="pmm")
                    m2h, m2o = divmod(m2, KI)
                    for ki in range(KI):
                        nc.tensor.matmul(
                            pd[:], dslab[m2h][ki][:, m2o * 128:(m2o + 1) * 128],
                            at[ki][:],
                            start=(ki == 0), stop=(ki == KI - 1))
                    ev = ev_pool.tile([128, n_tok], f32, tag="ev")
                    nc.vector.tensor_copy(ev[:], pd[:])
                    nc.sync.dma_start(
                        y_ap[m2 * 128:(m2 + 1) * 128, y_off:y_off + n_tok],
                        ev[:])

            swiglu_ffn(
                xsb, TOK_BLK, 0,
                lambda k, h: wsgu_d[k * 128:(k + 1) * 128,
                                    h * I:(h + 1) * I],
                lambda ki, h: wsd_d[ki * 128:(ki + 1) * 128,
                                    h * I:(h + 1) * I],
                ys_d, 0)
            xgs = emit_scores()
            for j in range(2):
                swiglu_ffn(
                    xgs, cap, j * cap,
                    lambda k, h, j=j: wgu_d[j, k * 128:(k + 1) * 128,
                                            h * I:(h + 1) * I],
                    lambda ki, h, j=j: wd_d[j, ki * 128:(ki + 1) * 128,
                                            h * I:(h + 1) * I],
                    yr_d, j * cap)

    nc.compile()
    return nc


def _plan(hidden_states, router_w):
    logits = hidden_states.astype(np.float64) @ router_w.astype(np.float64).T
    top = np.argmax(logits, axis=1)
    counts = np.bincount(top, minlength=E)
    order = np.argsort(-counts, kind="stable")
    pairs = [(int(order[i]), int(order[E - 1 - i])) for i in range(NCORES)]
    cap = int(np.ceil(max(1, counts.max()) / 8) * 8)
    idx = [np.nonzero(top == e)[0] for e in range(E)]
    return pairs, cap, idx


def _run(inputs, trace=False):
    from concourse.bass_utils import run_bass_kernel_spmd

    x = np.asarray(inputs["hidden_states"], dtype=np.float32)
    router_w = np.asarray(inputs["router_w"], dtype=np.float32)
    gup = np.asarray(inputs["gate_up_proj"], dtype=np.float32)
    dp = np.asarray(inputs["down_proj"], dtype=np.float32)
    sgu = np.asarray(inputs["shared_gate_up_w"], dtype=np.float32)
    sd = np.asarray(inputs["shared_down_w"], dtype=np.float32)

    pairs, cap, idx = _plan(x, router_w)
    NR = 2 * cap

    key = cap
    if key not in _PROG_CACHE:
        _PROG_CACHE[key] = _build_program(cap)
    nc = _PROG_CACHE[key]

    xT = np.ascontiguousarray(x.T)                     # [H, T] f32
    xT_bf = xT.astype(BF16)
    wsgu_t = np.ascontiguousarray(sgu.T).astype(BF16)  # [H, 2I]
    wsd_t = np.ascontiguousarray(sd.T).astype(BF16)    # [I, H]

    in_maps = []
    for c in range(NCORES):
        eA, eB = pairs[c]
        xg = np.zeros((H, NR), dtype=BF16)
        for j, e in enumerate((eA, eB)):
            ids = idx[e]
            xg[:, j * cap: j * cap + len(ids)] = xT_bf[:, ids]
        rw = np.stack([router_w[eA], router_w[eB]], axis=1)  # [H, 2]
        rw = rw.reshape(KH, 128, 2).transpose(1, 0, 2).reshape(128, 2 * KH)
        in_maps.append({
            "xg": xg,
            "rw": np.ascontiguousarray(rw).astype(BF16),
            "wgu": gup[[eA, eB]].astype(BF16),
            "wd": dp[[eA, eB]].astype(BF16),
            "xs": np.ascontiguousarray(
                xT_bf[:, c * TOK_BLK:(c + 1) * TOK_BLK]),
            "wsgu": wsgu_t,
            "wsd": wsd_t,
        })

    res = run_bass_kernel_spmd(nc, in_maps, core_ids=list(range(NCORES)),
                               trace=trace)

    out = np.empty((T, H), dtype=np.float32)
    for c in range(NCORES):
        out[c * TOK_BLK:(c + 1) * TOK_BLK] = res.results[c]["ys"].T
    for c in range(NCORES):
        yr = res.results[c]["yr"]
        for j, e in enumerate(pairs[c]):
            ids = idx[e]
            out[ids] += yr[:, j * cap: j * cap + len(ids)].T
    return out, res.exec_time_ns


def kernel(**inputs) -> np.ndarray:
    out, _ = _run(inputs, trace=False)
    return out


# revision 13
# speedup vs baseline: 1.0909x; 1.0256x over previous
"""Llama4TextMoe (T=4096, H=2048, I=1024, E=16, top-1) on 8 trn2 NeuronCores.

Strategy (expert-parallel, sparse dispatch):
  - Host: router argmax (integer routing decision only), balanced expert->core
    pairing (2 experts/core), gather of each expert's tokens, transposition to
    feature-major layout, bf16 casts, final scatter-add combine.
  - Device (SPMD, same program, per-core data): router scores
    (matvec + sigmoid), score scaling, SwiGLU expert FFN for the core's two
    experts over only their routed tokens, plus the shared-expert SwiGLU for
    this core's 512-token block. All matmuls bf16 with fp32 PSUM accumulation,
    feature-major activations so weights stream as natural-layout lhsT tiles.
  - Combine: out[block] = shared_block.T ; out[idx_e] += routed_e.T (host).
"""

import os
import sys

import numpy as np
import ml_dtypes

for _p in ("/opt/trn_rl_repo", "/root/.axon_site/_ro/trn_rl_repo"):
    if _p not in sys.path and os.path.isdir(_p):
        sys.path.append(_p)

T, H, I, E = 4096, 2048, 1024, 16
NCORES = 8
TOK_BLK = T // NCORES  # 512
KH = H // 128   # 16 contraction tiles over H
KI = I // 128   # 8  contraction tiles over I
MF = (2 * I) // 128  # 16 output tiles over gate_up ff dim
MH = H // 128   # 16 output tiles over H
BF16 = ml_dtypes.bfloat16

_PROG_CACHE: dict = {}


def _chunks(total, step):
    out = []
    st = 0
    while st < total:
        w = min(step, total - st)
        out.append((st, w))
        st += w
    return out


def _build_program(cap):
    import concourse.mybir as mybir
    import concourse.tile as tile
    from concourse import bacc

    f32 = mybir.dt.float32
    bf16 = mybir.dt.bfloat16
    AF = mybir.ActivationFunctionType
    MULT = mybir.AluOpType.mult
    NR = 2 * cap
    assert cap <= 512, f"per-expert capacity {cap} exceeds one PSUM bank"

    nc = bacc.Bacc("TRN2", target_bir_lowering=False, debug=False,
                   num_devices=NCORES)
    xg_d = nc.dram_tensor("xg", [H, NR], bf16, kind="ExternalInput").ap()
    rw_d = nc.dram_tensor("rw", [128, 2 * KH], bf16, kind="ExternalInput").ap()
    wgu_d = nc.dram_tensor("wgu", [2, H, 2 * I], bf16, kind="ExternalInput").ap()
    wd_d = nc.dram_tensor("wd", [2, I, H], bf16, kind="ExternalInput").ap()
    xs_d = nc.dram_tensor("xs", [H, TOK_BLK], bf16, kind="ExternalInput").ap()
    wsgu_d = nc.dram_tensor("wsgu", [H, 2 * I], bf16, kind="ExternalInput").ap()
    wsd_d = nc.dram_tensor("wsd", [I, H], bf16, kind="ExternalInput").ap()
    yr_d = nc.dram_tensor("yr", [H, NR], f32, kind="ExternalOutput").ap()
    ys_d = nc.dram_tensor("ys", [H, TOK_BLK], f32, kind="ExternalOutput").ap()

    with tile.TileContext(nc) as tc:
        with (
            tc.tile_pool(name="xgb", bufs=KH) as xgb_pool,
            tc.tile_pool(name="xgs", bufs=KH) as xgs_pool,
            tc.tile_pool(name="xsb", bufs=KH) as xsb_pool,
            tc.tile_pool(name="wslab", bufs=50) as w_pool,
            tc.tile_pool(name="act", bufs=10) as act_pool,
            tc.tile_pool(name="ga", bufs=8) as ga_pool,
            tc.tile_pool(name="up", bufs=2) as up_pool,
            tc.tile_pool(name="ev", bufs=5) as ev_pool,
            tc.tile_pool(name="small", bufs=2) as small_pool,
            tc.tile_pool(name="pmm", bufs=4, space="PSUM") as pmm_pool,
            tc.tile_pool(name="psml", bufs=2, space="PSUM") as psml_pool,
        ):
            # --- router weights + ones (for partition-broadcast matmul) ---
            rwt = small_pool.tile([128, 2 * KH], bf16, tag="rwt")
            nc.sync.dma_start(rwt[:], rw_d[:])
            ones = small_pool.tile([1, 128], bf16, tag="ones")
            nc.gpsimd.memset(ones[:], 1.0)

            # --- shared-expert token block + first gate_up weight half,
            #     interleaved so the first FFN matmul's inputs land ASAP ---
            xsb = []
            sgu0 = []
            for k in range(KH):
                t = w_pool.tile([128, I], bf16, tag="wslab")
                nc.sync.dma_start(t[:], wsgu_d[k * 128:(k + 1) * 128, 0:I])
                sgu0.append(t)
                t2 = xsb_pool.tile([128, TOK_BLK], bf16, tag="xsb")
                nc.sync.dma_start(t2[:], xs_d[k * 128:(k + 1) * 128, :])
                xsb.append(t2)

            def emit_scores():
                # --- gathered routed tokens, feature-major [H, NR] bf16 ---
                xgb = []
                for k in range(KH):
                    t = xgb_pool.tile([128, NR], bf16, tag="xgb")
                    nc.sync.dma_start(t[:], xg_d[k * 128:(k + 1) * 128, :])
                    xgb.append(t)

                # --- scores: s[j] = sigmoid(router_w[e_j] . x_t) ---
                s_sb = small_pool.tile([1, NR], bf16, tag="s_sb")
                for j in range(2):
                    ps = psml_pool.tile([1, cap], f32, tag="ps_score")
                    for k in range(KH):
                        nc.tensor.matmul(
                            ps[:], rwt[:, 2 * k + j:2 * k + j + 1],
                            xgb[k][:, j * cap:(j + 1) * cap],
                            start=(k == 0), stop=(k == KH - 1))
                    nc.scalar.activation(s_sb[:, j * cap:(j + 1) * cap],
                                         ps[:], AF.Sigmoid)

                # --- broadcast scores over 128 partitions via ones-matmul ---
                sbc = small_pool.tile([128, NR], bf16, tag="sbc")
                for (st, w) in _chunks(NR, 512):
                    pb = psml_pool.tile([128, w], f32, tag="pb")
                    nc.tensor.matmul(pb[:], ones[:], s_sb[:, st:st + w],
                                     start=True, stop=True)
                    nc.vector.tensor_copy(sbc[:, st:st + w], pb[:])

                # --- routed_in = x * score (both segments at once) ---
                xgs = []
                for k in range(KH):
                    t = xgs_pool.tile([128, NR], bf16, tag="xgs")
                    nc.vector.tensor_tensor(t[:], xgb[k][:], sbc[:], MULT)
                    xgs.append(t)
                return xgs

            # Weight slabs are loaded as column-halves ([128, 1024]): the
            # first half's last use is m-tile 7, so it frees mid-phase and
            # the NEXT phase's weight DMAs start that much earlier.
            def swiglu_ffn(x_tiles, n_tok, seg_off, gu_src, d_src, y_ap,
                           y_off, gu0=None, split_last=False):
                """gate_up -> silu*up -> down, feature-major, N=n_tok tokens."""
                guslab = [[None] * KH for _ in range(2)]
                if gu0 is not None:
                    guslab[0] = list(gu0)
                for h in range(2):
                    if h == 0 and gu0 is not None:
                        continue
                    for k in range(KH):
                        t = w_pool.tile([128, I], bf16, tag="wslab")
                        nc.sync.dma_start(t[:], gu_src(k, h))
                        guslab[h][k] = t
                ga = [None] * KI
                at = [None] * KI
                for m in range(MF):
                    pg = pmm_pool.tile([128, n_tok], f32, tag="pmm")
                    mh, mo = divmod(m, KI)
                    for k in range(KH):
                        nc.tensor.matmul(
                            pg[:], guslab[mh][k][:, mo * 128:(mo + 1) * 128],
                            x_tiles[k][:, seg_off:seg_off + n_tok],
                            start=(k == 0), stop=(k == KH - 1))
                    if m < KI:  # gate half
                        g = ga_pool.tile([128, n_tok], bf16, tag="ga")
                        nc.scalar.activation(g[:], pg[:], AF.Silu)
                        ga[m] = g
                    else:       # up half -> act = silu(gate) * up
                        u = up_pool.tile([128, n_tok], bf16, tag="up")
                        nc.vector.tensor_copy(u[:], pg[:])
                        a = act_pool.tile([128, n_tok], bf16, tag="at")
                        nc.vector.tensor_tensor(a[:], ga[m - KI][:], u[:], MULT)
                        at[m - KI] = a
                dslab = [[None] * KI for _ in range(2)]
                for h in range(2):
                    for ki in range(KI):
                        t = w_pool.tile([128, I], bf16, tag="wslab")
                        nc.sync.dma_start(t[:], d_src(ki, h))
                        dslab[h][ki] = t
                for m2 in range(MH):
                    pd = pmm_pool.tile([128, n_tok], f32, tag="pmm")
                    m2h, m2o = divmod(m2, KI)
                    for ki in range(KI):
                        nc.tensor.matmul(
                            pd[:], dslab[m2h][ki][:, m2o * 128:(m2o + 1) * 128],
                            at[ki][:],
                            start=(ki == 0), stop=(ki == KI - 1))
                    ev = ev_pool.tile([128, n_tok], f32, tag="ev")
                    nc.vector.tensor_copy(ev[:], pd[:])
                    row = y_ap[m2 * 128:(m2 + 1) * 128, :]
                    if split_last and m2 >= MH - 4:
                        half = n_tok // 2
                        nc.sync.dma_start(
                            row[:, y_off:y_off + half], ev[:, :half])
                        nc.sync.dma_start(
                            row[:, y_off + half:y_off + n_tok], ev[:, half:])
                    else:
                        nc.sync.dma_start(
                            row[:, y_off:y_off + n_tok], ev[:])

            swiglu_ffn(
                xsb, TOK_BLK, 0,
                lambda k, h: wsgu_d[k * 128:(k + 1) * 128,
                                    h * I:(h + 1) * I],
                lambda ki, h: wsd_d[ki * 128:(ki + 1) * 128,
                                    h * I:(h + 1) * I],
                ys_d, 0, gu0=sgu0)
            xgs = emit_scores()
            for j in range(2):
                swiglu_ffn(
                    xgs, cap, j * cap,
                    lambda k, h, j=j: wgu_d[j, k * 128:(k + 1) * 128,
                                            h * I:(h + 1) * I],
                    lambda ki, h, j=j: wd_d[j, ki * 128:(ki + 1) * 128,
                                            h * I:(h + 1) * I],
                    yr_d, j * cap, split_last=(j == 1))

    nc.compile()
    return nc


def _plan(hidden_states, router_w):
    logits = hidden_states.astype(np.float64) @ router_w.astype(np.float64).T
    top = np.argmax(logits, axis=1)
    counts = np.bincount(top, minlength=E)
    order = np.argsort(-counts, kind="stable")
    pairs = [(int(order[i]), int(order[E - 1 - i])) for i in range(NCORES)]
    cap = int(np.ceil(max(1, counts.max()) / 8) * 8)
    idx = [np.nonzero(top == e)[0] for e in range(E)]
    return pairs, cap, idx


def _run(inputs, trace=False):
    from concourse.bass_utils import run_bass_kernel_spmd

    x = np.asarray(inputs["hidden_states"], dtype=np.float32)
    router_w = np.asarray(inputs["router_w"], dtype=np.float32)
    gup = np.asarray(inputs["gate_up_proj"], dtype=np.float32)
    dp = np.asarray(inputs["down_proj"], dtype=np.float32)
    sgu = np.asarray(inputs["shared_gate_up_w"], dtype=np.float32)
    sd = np.asarray(inputs["shared_down_w"], dtype=np.float32)

    pairs, cap, idx = _plan(x, router_w)
    NR = 2 * cap

    key = cap
    if key not in _PROG_CACHE:
        _PROG_CACHE[key] = _build_program(cap)
    nc = _PROG_CACHE[key]

    xT = np.ascontiguousarray(x.T)                     # [H, T] f32
    xT_bf = xT.astype(BF16)
    wsgu_t = np.ascontiguousarray(sgu.T).astype(BF16)  # [H, 2I]
    wsd_t = np.ascontiguousarray(sd.T).astype(BF16)    # [I, H]

    in_maps = []
    for c in range(NCORES):
        eA, eB = pairs[c]
        xg = np.zeros((H, NR), dtype=BF16)
        for j, e in enumerate((eA, eB)):
            ids = idx[e]
            xg[:, j * cap: j * cap + len(ids)] = xT_bf[:, ids]
        rw = np.stack([router_w[eA], router_w[eB]], axis=1)  # [H, 2]
        rw = rw.reshape(KH, 128, 2).transpose(1, 0, 2).reshape(128, 2 * KH)
        in_maps.append({
            "xg": xg,
            "rw": np.ascontiguousarray(rw).astype(BF16),
            "wgu": gup[[eA, eB]].astype(BF16),
            "wd": dp[[eA, eB]].astype(BF16),
            "xs": np.ascontiguousarray(
                xT_bf[:, c * TOK_BLK:(c + 1) * TOK_BLK]),
            "wsgu": wsgu_t,
            "wsd": wsd_t,
        })

    res = run_bass_kernel_spmd(nc, in_maps, core_ids=list(range(NCORES)),
                               trace=trace)

    out = np.empty((T, H), dtype=np.float32)
    for c in range(NCORES):
        out[c * TOK_BLK:(c + 1) * TOK_BLK] = res.results[c]["ys"].T
    for c in range(NCORES):
        yr = res.results[c]["yr"]
        for j, e in enumerate(pairs[c]):
            ids = idx[e]
            out[ids] += yr[:, j * cap: j * cap + len(ids)].T
    return out, res.exec_time_ns


def kernel(**inputs) -> np.ndarray:
    out, _ = _run(inputs, trace=False)
    return out


# revision 14
# speedup vs baseline: 1.1365x; 1.0418x over previous
"""Llama4TextMoe (T=4096, H=2048, I=1024, E=16, top-1) on 8 trn2 NeuronCores.

Strategy (expert-parallel, sparse dispatch):
  - Host: router argmax (integer routing decision only), balanced expert->core
    pairing (2 experts/core), gather of each expert's tokens, transposition to
    feature-major layout, bf16 casts, final scatter-add combine.
  - Device (SPMD, same program, per-core data): router scores
    (matvec + sigmoid), score scaling, SwiGLU expert FFN for the core's two
    experts over only their routed tokens, plus the shared-expert SwiGLU for
    this core's 512-token block. All matmuls bf16 with fp32 PSUM accumulation,
    feature-major activations so weights stream as natural-layout lhsT tiles.
  - Combine: out[block] = shared_block.T ; out[idx_e] += routed_e.T (host).
"""

import os
import sys

import numpy as np
import ml_dtypes

for _p in ("/opt/trn_rl_repo", "/root/.axon_site/_ro/trn_rl_repo"):
    if _p not in sys.path and os.path.isdir(_p):
        sys.path.append(_p)

T, H, I, E = 4096, 2048, 1024, 16
NCORES = 8
TOK_BLK = T // NCORES  # 512
KH = H // 128   # 16 contraction tiles over H
KI = I // 128   # 8  contraction tiles over I
MF = (2 * I) // 128  # 16 output tiles over gate_up ff dim
MH = H // 128   # 16 output tiles over H
BF16 = ml_dtypes.bfloat16

_PROG_CACHE: dict = {}


def _chunks(total, step):
    out = []
    st = 0
    while st < total:
        w = min(step, total - st)
        out.append((st, w))
        st += w
    return out


def _build_program(capA, capB):
    import concourse.mybir as mybir
    import concourse.tile as tile
    from concourse import bacc

    f32 = mybir.dt.float32
    bf16 = mybir.dt.bfloat16
    AF = mybir.ActivationFunctionType
    MULT = mybir.AluOpType.mult
    NR = capA + capB
    SEGS = ((0, capA), (capA, capB))
    assert max(capA, capB) <= 512, f"capacity {(capA, capB)} exceeds PSUM bank"

    nc = bacc.Bacc("TRN2", target_bir_lowering=False, debug=False,
                   num_devices=NCORES)
    xg_d = nc.dram_tensor("xg", [H, NR], bf16, kind="ExternalInput").ap()
    rw_d = nc.dram_tensor("rw", [128, 2 * KH], bf16, kind="ExternalInput").ap()
    wgu_d = nc.dram_tensor("wgu", [2, H, 2 * I], bf16, kind="ExternalInput").ap()
    wd_d = nc.dram_tensor("wd", [2, I, H], bf16, kind="ExternalInput").ap()
    xs_d = nc.dram_tensor("xs", [H, TOK_BLK], bf16, kind="ExternalInput").ap()
    wsgu_d = nc.dram_tensor("wsgu", [H, 2 * I], bf16, kind="ExternalInput").ap()
    wsd_d = nc.dram_tensor("wsd", [I, H], bf16, kind="ExternalInput").ap()
    yr_d = nc.dram_tensor("yr", [H, NR], f32, kind="ExternalOutput").ap()
    ys_d = nc.dram_tensor("ys", [H, TOK_BLK], f32, kind="ExternalOutput").ap()

    with tile.TileContext(nc) as tc:
        with (
            tc.tile_pool(name="xgb", bufs=KH) as xgb_pool,
            tc.tile_pool(name="xgs", bufs=KH) as xgs_pool,
            tc.tile_pool(name="xsb", bufs=KH) as xsb_pool,
            tc.tile_pool(name="wslab", bufs=50) as w_pool,
            tc.tile_pool(name="act", bufs=10) as act_pool,
            tc.tile_pool(name="ga", bufs=8) as ga_pool,
            tc.tile_pool(name="up", bufs=2) as up_pool,
            tc.tile_pool(name="ev", bufs=5) as ev_pool,
            tc.tile_pool(name="small", bufs=2) as small_pool,
            tc.tile_pool(name="pmm", bufs=4, space="PSUM") as pmm_pool,
            tc.tile_pool(name="psml", bufs=2, space="PSUM") as psml_pool,
        ):
            # --- router weights + ones (for partition-broadcast matmul) ---
            rwt = small_pool.tile([128, 2 * KH], bf16, tag="rwt")
            nc.sync.dma_start(rwt[:], rw_d[:])
            ones = small_pool.tile([1, 128], bf16, tag="ones")
            nc.gpsimd.memset(ones[:], 1.0)

            # --- shared-expert token block + first gate_up weight half,
            #     interleaved so the first FFN matmul's inputs land ASAP ---
            xsb = []
            sgu0 = []
            for k in range(KH):
                t = w_pool.tile([128, I], bf16, tag="wslab")
                nc.sync.dma_start(t[:], wsgu_d[k * 128:(k + 1) * 128, 0:I])
                sgu0.append(t)
                t2 = xsb_pool.tile([128, TOK_BLK], bf16, tag="xsb")
                nc.gpsimd.dma_start(t2[:], xs_d[k * 128:(k + 1) * 128, :])
                xsb.append(t2)

            def emit_scores():
                # --- gathered routed tokens, feature-major [H, NR] bf16 ---
                xgb = []
                for k in range(KH):
                    t = xgb_pool.tile([128, NR], bf16, tag="xgb")
                    nc.sync.dma_start(t[:], xg_d[k * 128:(k + 1) * 128, :])
                    xgb.append(t)

                # --- scores: s[j] = sigmoid(router_w[e_j] . x_t) ---
                s_sb = small_pool.tile([1, NR], bf16, tag="s_sb")
                for j, (so, sn) in enumerate(SEGS):
                    ps = psml_pool.tile([1, max(capA, capB)], f32,
                                        tag="ps_score")
                    for k in range(KH):
                        nc.tensor.matmul(
                            ps[:, :sn], rwt[:, 2 * k + j:2 * k + j + 1],
                            xgb[k][:, so:so + sn],
                            start=(k == 0), stop=(k == KH - 1))
                    nc.scalar.activation(s_sb[:, so:so + sn],
                                         ps[:, :sn], AF.Sigmoid)

                # --- broadcast scores over 128 partitions via ones-matmul ---
                sbc = small_pool.tile([128, NR], bf16, tag="sbc")
                for (st, w) in _chunks(NR, 512):
                    pb = psml_pool.tile([128, w], f32, tag="pb")
                    nc.tensor.matmul(pb[:], ones[:], s_sb[:, st:st + w],
                                     start=True, stop=True)
                    nc.vector.tensor_copy(sbc[:, st:st + w], pb[:])

                # --- routed_in = x * score (both segments at once) ---
                xgs = []
                for k in range(KH):
                    t = xgs_pool.tile([128, NR], bf16, tag="xgs")
                    nc.vector.tensor_tensor(t[:], xgb[k][:], sbc[:], MULT)
                    xgs.append(t)
                return xgs

            # Weight slabs are loaded as column-halves ([128, 1024]): the
            # first half's last use is m-tile 7, so it frees mid-phase and
            # the NEXT phase's weight DMAs start that much earlier.
            def swiglu_ffn(x_tiles, n_tok, seg_off, gu_src, d_src, y_ap,
                           y_off, gu0=None, split_last=False):
                """gate_up -> silu*up -> down, feature-major, N=n_tok tokens."""
                guslab = [[None] * KH for _ in range(2)]
                if gu0 is not None:
                    guslab[0] = list(gu0)
                for h in range(2):
                    if h == 0 and gu0 is not None:
                        continue
                    for k in range(KH):
                        t = w_pool.tile([128, I], bf16, tag="wslab")
                        nc.sync.dma_start(t[:], gu_src(k, h))
                        guslab[h][k] = t
                ga = [None] * KI
                at = [None] * KI
                for m in range(MF):
                    pg = pmm_pool.tile([128, n_tok], f32, tag="pmm")
                    mh, mo = divmod(m, KI)
                    for k in range(KH):
                        nc.tensor.matmul(
                            pg[:], guslab[mh][k][:, mo * 128:(mo + 1) * 128],
                            x_tiles[k][:, seg_off:seg_off + n_tok],
                            start=(k == 0), stop=(k == KH - 1))
                    if m < KI:  # gate half
                        g = ga_pool.tile([128, n_tok], bf16, tag="ga")
                        nc.scalar.activation(g[:], pg[:], AF.Silu)
                        ga[m] = g
                    else:       # up half -> act = silu(gate) * up
                        u = up_pool.tile([128, n_tok], bf16, tag="up")
                        nc.vector.tensor_copy(u[:], pg[:])
                        a = act_pool.tile([128, n_tok], bf16, tag="at")
                        nc.vector.tensor_tensor(a[:], ga[m - KI][:], u[:], MULT)
                        at[m - KI] = a
                dslab = [[None] * KI for _ in range(2)]
                for h in range(2):
                    for ki in range(KI):
                        t = w_pool.tile([128, I], bf16, tag="wslab")
                        nc.sync.dma_start(t[:], d_src(ki, h))
                        dslab[h][ki] = t
                for m2 in range(MH):
                    pd = pmm_pool.tile([128, n_tok], f32, tag="pmm")
                    m2h, m2o = divmod(m2, KI)
                    for ki in range(KI):
                        nc.tensor.matmul(
                            pd[:], dslab[m2h][ki][:, m2o * 128:(m2o + 1) * 128],
                            at[ki][:],
                            start=(ki == 0), stop=(ki == KI - 1))
                    ev = ev_pool.tile([128, n_tok], f32, tag="ev")
                    nc.vector.tensor_copy(ev[:], pd[:])
                    row = y_ap[m2 * 128:(m2 + 1) * 128, :]
                    if split_last and m2 >= MH - 4:
                        half = n_tok // 2
                        nc.sync.dma_start(
                            row[:, y_off:y_off + half], ev[:, :half])
                        nc.sync.dma_start(
                            row[:, y_off + half:y_off + n_tok], ev[:, half:])
                    else:
                        nc.sync.dma_start(
                            row[:, y_off:y_off + n_tok], ev[:])

            swiglu_ffn(
                xsb, TOK_BLK, 0,
                lambda k, h: wsgu_d[k * 128:(k + 1) * 128,
                                    h * I:(h + 1) * I],
                lambda ki, h: wsd_d[ki * 128:(ki + 1) * 128,
                                    h * I:(h + 1) * I],
                ys_d, 0, gu0=sgu0)
            xgs = emit_scores()
            for j, (so, sn) in enumerate(SEGS):
                swiglu_ffn(
                    xgs, sn, so,
                    lambda k, h, j=j: wgu_d[j, k * 128:(k + 1) * 128,
                                            h * I:(h + 1) * I],
                    lambda ki, h, j=j: wd_d[j, ki * 128:(ki + 1) * 128,
                                            h * I:(h + 1) * I],
                    yr_d, so, split_last=(j == 1))

    nc.compile()
    return nc


def _plan(hidden_states, router_w):
    logits = hidden_states.astype(np.float64) @ router_w.astype(np.float64).T
    top = np.argmax(logits, axis=1)
    counts = np.bincount(top, minlength=E)
    order = np.argsort(-counts, kind="stable")
    big, small = order[:NCORES], order[NCORES:]
    # slot A holds the 8 largest experts, slot B the 8 smallest; pair
    # largest-with-smallest for per-core load balance
    pairs = [(int(big[i]), int(small[NCORES - 1 - i])) for i in range(NCORES)]
    r8 = lambda n: int(np.ceil(max(1, n) / 8) * 8)
    capA = r8(counts[big].max())
    capB = r8(counts[small].max())
    idx = [np.nonzero(top == e)[0] for e in range(E)]
    return pairs, capA, capB, idx


def _run(inputs, trace=False):
    from concourse.bass_utils import run_bass_kernel_spmd

    x = np.asarray(inputs["hidden_states"], dtype=np.float32)
    router_w = np.asarray(inputs["router_w"], dtype=np.float32)
    gup = np.asarray(inputs["gate_up_proj"], dtype=np.float32)
    dp = np.asarray(inputs["down_proj"], dtype=np.float32)
    sgu = np.asarray(inputs["shared_gate_up_w"], dtype=np.float32)
    sd = np.asarray(inputs["shared_down_w"], dtype=np.float32)

    pairs, capA, capB, idx = _plan(x, router_w)
    NR = capA + capB
    segs = ((0, capA), (capA, capB))

    key = (capA, capB)
    if key not in _PROG_CACHE:
        _PROG_CACHE[key] = _build_program(capA, capB)
    nc = _PROG_CACHE[key]

    xT = np.ascontiguousarray(x.T)                     # [H, T] f32
    xT_bf = xT.astype(BF16)
    wsgu_t = np.ascontiguousarray(sgu.T).astype(BF16)  # [H, 2I]
    wsd_t = np.ascontiguousarray(sd.T).astype(BF16)    # [I, H]

    in_maps = []
    for c in range(NCORES):
        eA, eB = pairs[c]
        xg = np.zeros((H, NR), dtype=BF16)
        for (so, sn), e in zip(segs, (eA, eB)):
            ids = idx[e]
            xg[:, so: so + len(ids)] = xT_bf[:, ids]
        rw = np.stack([router_w[eA], router_w[eB]], axis=1)  # [H, 2]
        rw = rw.reshape(KH, 128, 2).transpose(1, 0, 2).reshape(128, 2 * KH)
        in_maps.append({
            "xg": xg,
            "rw": np.ascontiguousarray(rw).astype(BF16),
            "wgu": gup[[eA, eB]].astype(BF16),
            "wd": dp[[eA, eB]].astype(BF16),
            "xs": np.ascontiguousarray(
                xT_bf[:, c * TOK_BLK:(c + 1) * TOK_BLK]),
            "wsgu": wsgu_t,
            "wsd": wsd_t,
        })

    res = run_bass_kernel_spmd(nc, in_maps, core_ids=list(range(NCORES)),
                               trace=trace)

    out = np.empty((T, H), dtype=np.float32)
    for c in range(NCORES):
        out[c * TOK_BLK:(c + 1) * TOK_BLK] = res.results[c]["ys"].T
    for c in range(NCORES):
        yr = res.results[c]["yr"]
        for (so, sn), e in zip(segs, pairs[c]):
            ids = idx[e]
            out[ids] += yr[:, so: so + len(ids)].T
    return out, res.exec_time_ns


def kernel(**inputs) -> np.ndarray:
    out, _ = _run(inputs, trace=False)
    return out
